# revision 17
# baseline (speedup 1.0000x reference)
"""HNetv1 Trainium2 Bass kernel — v2 (fp8 everywhere on the hot path).

Strategy (8 NeuronCores):
  - Inputs x1/x2 land as fp8e4 (no scale; randn fits e4m3). Every core
    computes the full correlation for all 64 batches (needed as the
    activation input for the tensor-parallel L1).
  - Correlation pipeline per group of G=8 batches:
      gpsimd: sq1 = x1*x1 (bf16)     scalar: sq2 = square(x2) (bf16)
      PE: col-tiled ones-matmuls reduce sum_c(sq) into a compact 4-strip
          psum layout [32q+p, f]; stationary consts 4.0/16.0 pre-fold the
          per-side scale so sqrt(recip(ssq)) directly yields 16/32*rsqrt
          and 8/32*rsqrt (the 1/32 is the row-tiled broadcast redundancy).
      DVE: reciprocal_approx_fast (compact);  scalar: sqrt -> bf16.
      PE: row-tiled ones-matmuls broadcast r to all 128 partitions (x32).
      DVE: x1n = x1*r1b, x2n = x2*r2b (bf16; scale 16 resp 8 folded in so
          corr psum = 128*corr_normalized, ideal fp8 range).
      PE: per batch, corr main [c,128ij]x[c,144k] -> ca[ij,(b,k)]; the
          16-row ij-residue is computed OPERAND-SWAPPED out[k,(b,ijr)]
          (plus a 16x16 corner) so drains stay 128-lane-parallel.
      drains: scalar Copy ca -> X_a fp8 [ij,k,n]; DVE copies for the
          swapped-residue [k,ijr,n] and corner.
  - L1 (x[64,20736] @ w1-slice[20736,648]) is column-split 8 ways and runs
    as 81 Double-FP8 (DoubleRow) pair-tile matmuls: lhsT = X pair
    [128,2,64], rhs = w1 pair [128,2,648] (fp8, host-scaled x1024, pair
    stride padded to 656 B for the %16 ISA rule). w1 streams from HBM in 9
    contiguous ~1.5 MB chunks issued up front on the scalar HWDGE ring.
  - L2 is row-split: h1 is transposed to fp8 pairs [128,6,64] (drain scale
    2^-9) and multiplied with fp8 w2 (x1024) in 9 DoubleRow matmuls; the
    2^-18 descale happens in the psum->bf16 drain. One AllToAll exchanges
    batch-shards of the [64,1296] partials; each core sums its 8 shards
    with sel-matmuls (b2 pre-divided by 8 on host, added at 2^18 scale).
  - L3/L4 run on 8 batches/core in bf16 (baseline structure). The host
    concatenates the per-core [8,8] outputs.
  - A dummy AllGather fires at kernel start to absorb launch skew on the
    CC queue; a PE warm-up burst flips the HAM clock gate early (the ssq
    stationary consts are derived from its output to force scheduling).

Scale bookkeeping (all powers of 2, folded on host):
  X_fp8 = 128*corr_n;  w1_fp8 = 1024*w1  => psum = 2^17 * z1
  h1_fp8 = relu(psum)*2^-9 = 2^8 * h1;   w2_fp8 = 1024*w2 => psum = 2^18*z2
  h2 = psum*2^-18 + b2/8 (bias pre-scaled 2^18 into the ones-matmul row).
"""

import os
import numpy as np
import ml_dtypes

N, C, S = 64, 128, 12
HW = S * S            # 144
RIN = S ** 4          # 20736
NCORES = 8
COLS1 = 5184 // NCORES   # 648
PADC = 656               # pair stride (bytes, fp8) — 16-aligned
NPAIR = 81               # 162 k-tiles as DoubleRow pairs
G = 8                    # batches per corr group
NGRP = N // G            # 8
NB = N // NCORES         # 8
D2 = 1296
D3 = 324
PADK3 = 1408             # 1296 padded to 11*128
PADK4 = 384              # 324 padded to 3*128

_CACHE = {}
LAST_RESULT = None


def _bf16(a):
    return np.asarray(a, dtype=np.float32).astype(ml_dtypes.bfloat16)


def _fp8(a):
    return np.clip(np.asarray(a, dtype=np.float32), -240.0, 240.0).astype(
        ml_dtypes.float8_e4m3)


def _build_nc(trace_enabled=False):
    import concourse.bacc as bacc
    import concourse.tile as tile
    import concourse.mybir as mybir

    from concourse.masks import make_identity

    dt = mybir.dt
    AF = mybir.ActivationFunctionType
    ALU = mybir.AluOpType
    DR = mybir.MatmulPerfMode.DoubleRow

    nc = bacc.Bacc("TRN2", target_bir_lowering=False, debug=False,
                   num_devices=NCORES)

    x1q_d = nc.dram_tensor("x1q", [C, N, HW], dt.bfloat16, kind="ExternalInput")
    x2q_d = nc.dram_tensor("x2q", [C, N, HW], dt.bfloat16, kind="ExternalInput")
    w1m_d = nc.dram_tensor("w1m", [128, NPAIR * 2 * PADC], dt.float8e4,
                           kind="ExternalInput")
    b1s_d = nc.dram_tensor("b1s", [1, COLS1], dt.bfloat16, kind="ExternalInput")
    w2m_d = nc.dram_tensor("w2m", [128, 6 * D2], dt.float8e4,
                           kind="ExternalInput")
    b2f_d = nc.dram_tensor("b2f", [1, D2], dt.bfloat16, kind="ExternalInput")
    w3f_d = nc.dram_tensor("w3f", [PADK3, D3], dt.bfloat16, kind="ExternalInput")
    b3_d = nc.dram_tensor("b3r", [1, D3], dt.bfloat16, kind="ExternalInput")
    w4p_d = nc.dram_tensor("w4p", [PADK4, 8], dt.bfloat16, kind="ExternalInput")
    b4_d = nc.dram_tensor("b4r", [1, 8], dt.bfloat16, kind="ExternalInput")
    out_d = nc.dram_tensor("out", [NB, 8], dt.float32, kind="ExternalOutput")
    dbg = os.environ.get("HNET_DEBUG", "0") == "1"
    if dbg:
        dXa_d = nc.dram_tensor("dXa", [128, HW * N], dt.float8e4,
                               kind="ExternalOutput")
        dXsw_d = nc.dram_tensor("dXsw", [128, 16 * N], dt.float8e4,
                                kind="ExternalOutput")
        dXcp_d = nc.dram_tensor("dXcp", [128, 2 * N], dt.float8e4,
                                kind="ExternalOutput")
        drcb_d = nc.dram_tensor("drcb", [128, 576], dt.bfloat16,
                                kind="ExternalOutput")
        dh1_d = nc.dram_tensor("dh1", [64, COLS1], dt.bfloat16,
                               kind="ExternalOutput")
        dh2_d = nc.dram_tensor("dh2", [64, D2], dt.bfloat16,
                               kind="ExternalOutput")
        drbs_d = nc.dram_tensor("drbs", [128, 1152], dt.bfloat16,
                                kind="ExternalOutput")
        dXg0_d = nc.dram_tensor("dXg0", [128, G * HW], dt.float8e4,
                                kind="ExternalOutput")
        dx1n_d = nc.dram_tensor("dx1n", [128, 1152], dt.bfloat16,
                                kind="ExternalOutput")

    rg = [list(range(NCORES))]

    with tile.TileContext(nc) as tc:
        with tc.tile_pool(name="persist", bufs=1) as persist, \
             tc.tile_pool(name="dramp", bufs=1, space="DRAM") as dramp:
            a2a_in = dramp.tile([N, D2], dt.bfloat16)
            a2a_out = dramp.tile([N, D2], dt.bfloat16)
            sync_in = dramp.tile([1, 64], dt.bfloat16)
            sync_out = dramp.tile([NCORES, 64], dt.bfloat16, addr_space="Shared")

            # all corr input loads first in the sync-ring FIFO
            x1ts, x2ts = [], []
            for g in range(NGRP):
                n0 = G * g
                x1t = persist.tile([C, G, HW], dt.bfloat16, tag=f"x1t{g}")
                nc.sync.dma_start(x1t[:], x1q_d[:, n0:n0 + G, :])
                x1ts.append(x1t)
                x2t = persist.tile([C, G, HW], dt.bfloat16, tag=f"x2t{g}")
                nc.sync.dma_start(x2t[:], x2q_d[:, n0:n0 + G, :])
                x2ts.append(x2t)

            ones128 = persist.tile([128, 128], dt.bfloat16)
            nc.vector.memset(ones128[:], 1.0)
            onesrow = persist.tile([1, N], dt.bfloat16)
            nc.vector.memset(onesrow[:], 1.0)
            ident = persist.tile([128, 128], dt.bfloat16)
            make_identity(nc, ident[:])
            nc.sync.dma_start(sync_in[:, :], onesrow[:, 0:64])
            selT = persist.tile([64, 8], dt.bfloat16)
            for r in range(NCORES):
                nc.sync.dma_start(selT[8 * r:8 * r + 8, :], ident[0:8, 0:8])

            # big streaming weights: w1 fp8 in 9 contiguous chunks, then w2
            w1sb = persist.tile([128, NPAIR * 2 * PADC], dt.float8e4)
            CHB = 9 * 2 * PADC     # 9 pairs per chunk
            for ch in range(9):
                nc.scalar.dma_start(w1sb[:, CHB * ch:CHB * (ch + 1)],
                                    w1m_d[:, CHB * ch:CHB * (ch + 1)])
            w2sb = persist.tile([128, 6 * D2], dt.float8e4)
            nc.scalar.dma_start(w2sb[:], w2m_d[:, :])

            # PE warm-up (~5us) to flip the HAM clock gate; the corr ssq
            # stationary consts are derived from its output so it schedules
            # first.
            c4 = persist.tile([128, 32], dt.bfloat16)
            c16 = persist.tile([128, 32], dt.bfloat16)
            with tc.tile_pool(name="wub", bufs=1) as wub, \
                 tc.tile_pool(name="pwu", bufs=1, space="PSUM") as pwu:
                ones512 = wub.tile([128, 512], dt.bfloat16, tag="o512")
                nc.vector.memset(ones512[:], 1.0)
                wu = pwu.tile([128, 512], dt.float32, tag="wu")
                for _ in range(12):
                    nc.tensor.matmul(wu[:], ones128[:], ones512[:],
                                     start=True, stop=True)
                # ssq stationaries: 1/256 resp 1/64 so sqrt(recip(ssq*c))
                # yields 16*rsqrt (x1 side) resp 8*rsqrt (x2 side) directly
                nc.vector.tensor_scalar_mul(c4[:], wu[:, 0:32], 1.0 / 32768.0)
                nc.vector.tensor_scalar_mul(c16[:], wu[:, 32:64], 1.0 / 8192.0)

            # merged X: kappa 0:144 = main k-tiles [ij, n, k]; 144:160 =
            # swres tiles [k, n, ijr]. DoubleRow pairs at kappa distance 16.
            XM = persist.tile([128, N, 160], dt.float8e4)
            X_co = persist.tile([16, N, 16], dt.float8e4)     # [kr, n, ijrs]
            X_cp = persist.tile([128, 2, N], dt.float8e4)     # corner pair

            # ---------------- corr phase ----------------
            # per group: squares (gpsimd/scalar) -> col-tiled ssq matmuls
            # (compact 4-strip psum) -> DVE recip -> scalar sqrt (scale
            # pre-folded via the 4.0/16.0 stationaries) -> 16 small
            # partition-shifted DMAs replicate the strips into full-height
            # bf16 r tiles -> DVE 2x normalize mults -> corr matmuls ->
            # fp8 drains.
            with tc.tile_pool(name="csq", bufs=2) as csq, \
                 tc.tile_pool(name="crr", bufs=2) as crr, \
                 tc.tile_pool(name="crb", bufs=2) as crb, \
                 tc.tile_pool(name="cxn", bufs=2) as cxn, \
                 tc.tile_pool(name="pssq", bufs=1, space="PSUM") as pssq, \
                 tc.tile_pool(name="pca", bufs=1, space="PSUM") as pca, \
                 tc.tile_pool(name="pcr", bufs=1, space="PSUM") as pcr:
                for g in range(NGRP):
                    n0 = G * g
                    x1t, x2t = x1ts[g], x2ts[g]
                    x1f = x1t[:].rearrange("c b k -> c (b k)")
                    x2f = x2t[:].rearrange("c b k -> c (b k)")

                    sq1 = csq.tile([C, G * HW], dt.bfloat16, tag="sq1")
                    nc.gpsimd.tensor_tensor(sq1[:], x1f, x1f, ALU.mult)
                    if g == 0:
                        # launch-skew absorber behind the first square on
                        # the gpsimd queue
                        nc.gpsimd.collective_compute(
                            "AllGather", mybir.AluOpType.bypass,
                            replica_groups=rg,
                            ins=[sync_in[:]], outs=[sync_out[:]])
                    sq2 = csq.tile([C, G * HW], dt.bfloat16, tag="sq2")
                    nc.scalar.activation(sq2[:], x2f, AF.Square)

                    # ssq compact: strip q=2(side-1)+half holds fd
                    # [576q : 576q+576) at partitions 32q.., banks by h
                    pn = pssq.tile([128, 2, 512], dt.float32, tag="pn")
                    FQ = 288
                    for q in range(4):
                        sq = sq1 if q < 2 else sq2
                        cst = c4 if q < 2 else c16
                        base = 576 * (q % 2)
                        for h in range(2):
                            nc.tensor.matmul(
                                pn[32 * q:32 * q + 32, h, 0:FQ],
                                cst[:], sq[:, base + FQ * h:base + FQ * (h + 1)],
                                start=True, stop=True,
                                tile_position=(0, 32 * q))
                    rcpf = crr.tile([128, 576], dt.float32, tag="rcpf")
                    nc.vector.reciprocal_approx_fast(rcpf[:, 0:288],
                                                     pn[:, 0, 0:288])
                    nc.vector.reciprocal_approx_fast(rcpf[:, 288:576],
                                                     pn[:, 1, 0:288])
                    rcb = crr.tile([128, 576], dt.bfloat16, tag="rcb")
                    nc.scalar.activation(rcb[:], rcpf[:], AF.Sqrt)
                    if dbg and g == 0:
                        nc.sync.dma_start(drcb_d[:, :], rcb[:])

                    # replicate strips to full height (bf16, SBUF)
                    rbs1 = crb.tile([128, 1152], dt.bfloat16, tag="rbs1")
                    rbs2 = crb.tile([128, 1152], dt.bfloat16, tag="rbs2")
                    for (rbs, q0) in ((rbs1, 0), (rbs2, 2)):
                        for half in range(2):
                            q = q0 + half
                            for m in range(4):
                                nc.sync.dma_start(
                                    rbs[32 * m:32 * m + 32,
                                        576 * half:576 * half + 576],
                                    rcb[32 * q:32 * q + 32, :])

                    x1n = cxn.tile([C, G * HW], dt.bfloat16, tag="x1n")
                    nc.vector.tensor_tensor(x1n[:], x1f, rbs1[:], ALU.mult)
                    if dbg and g == 0:
                        nc.sync.dma_start(drbs_d[:, :], rbs1[:])
                        nc.sync.dma_start(dx1n_d[:, :], x1n[:])
                    x2n = cxn.tile([C, G * HW], dt.bfloat16, tag="x2n")
                    nc.vector.tensor_tensor(x2n[:], x2f, rbs2[:], ALU.mult)

                    # 3 batches per 512-fp32 psum bank
                    ca = pca.tile([128, 3, 512], dt.float32, tag="ca")
                    car = pcr.tile([128, 128], dt.float32, tag="car")
                    ccr = pcr.tile([16, 128], dt.float32, tag="ccr")
                    for b in range(G):
                        f0 = HW * b
                        nc.tensor.matmul(ca[:, b // 3,
                                            HW * (b % 3):HW * (b % 3) + HW],
                                         x1n[:, f0:f0 + 128],
                                         x2n[:, f0:f0 + HW],
                                         start=True, stop=True)
                        nc.tensor.matmul(car[:, 16 * b:16 * b + 16],
                                         x2n[:, f0:f0 + 128],
                                         x1n[:, f0 + 128:f0 + HW],
                                         start=True, stop=True)
                        nc.tensor.matmul(ccr[:, 16 * b:16 * b + 16],
                                         x2n[:, f0 + 128:f0 + HW],
                                         x1n[:, f0 + 128:f0 + HW],
                                         start=True, stop=True)
                    # drains (fp8): scalar banks 0-1, DVE bank 2 + residues
                    nc.scalar.activation(
                        XM[:, n0:n0 + 6, 0:HW].rearrange(
                            "p (B s) k -> p B s k", B=2),
                        ca[:, 0:2, 0:3 * HW].rearrange(
                            "p B (s k) -> p B s k", s=3),
                        AF.Copy)
                    nc.vector.tensor_copy(
                        XM[:, n0 + 6:n0 + 8, 0:HW],
                        ca[:, 2, 0:2 * HW].rearrange("p (s k) -> p s k", s=2))
                    nc.vector.tensor_copy(
                        XM[:, n0:n0 + G, HW:160],
                        car[:].rearrange("p (b r) -> p b r", b=G))
                    nc.vector.tensor_copy(
                        X_co[:, n0:n0 + G, :].rearrange("p b r -> p (b r)"),
                        ccr[:])
                    if dbg and g == 0:
                        nc.sync.dma_start(
                            dXg0_d[:].rearrange("p (b k) -> p b k", b=G),
                            XM[:, 0:G, 0:HW])

            # corner regroup: X_cp[16*il+kr, j, n] = X_co[kr, n, 8j+il]
            for il in range(8):
                for j in range(2):
                    nc.sync.dma_start(X_cp[16 * il:16 * il + 16, j, :],
                                      X_co[:, :, 8 * j + il])

            sync2_out = dramp.tile([NCORES, 64], dt.bfloat16,
                                   addr_space="Shared")
            nc.gpsimd.collective_compute(
                "AllGather", mybir.AluOpType.bypass, replica_groups=rg,
                ins=[sync_in[:]], outs=[sync2_out[:]])
            if dbg:
                nc.sync.dma_start(dXcp_d[:, :],
                                  X_cp[:].rearrange("p j n -> p (j n)"))
            # small weights for L3/L4 — load during L1
            w3sb = persist.tile([128, 11, D3], dt.bfloat16)
            nc.scalar.dma_start(w3sb[:], w3f_d[:].rearrange("(t p) c -> p t c", p=128))
            b3row = persist.tile([1, D3], dt.bfloat16)
            nc.scalar.dma_start(b3row[:], b3_d[:, :])
            w4sb = persist.tile([128, 3, 8], dt.bfloat16)
            nc.scalar.dma_start(w4sb[:], w4p_d[:].rearrange("(t p) c -> p t c", p=128))
            b4row = persist.tile([1, 8], dt.bfloat16)
            nc.scalar.dma_start(b4row[:], b4_d[:, :])
            b2row = persist.tile([1, D2], dt.bfloat16)
            nc.scalar.dma_start(b2row[:], b2f_d[:, :])

            # ---------------- L1: 81 DoubleRow pair matmuls ----------------
            h1sb = persist.tile([64, COLS1], dt.bfloat16)
            w1v = w1sb[:].rearrange("p (t j c) -> p t j c", t=NPAIR, j=2)
            with tc.tile_pool(name="bias", bufs=1) as biasp, \
                 tc.tile_pool(name="ph1", bufs=1, space="PSUM") as ph1:
                b1row = biasp.tile([1, COLS1], dt.bfloat16, tag="b1")
                nc.scalar.dma_start(b1row[:], b1s_d[:, :])
                h1ps = [ph1.tile([64, 324], dt.float32, tag=f"h1ps{h}",
                                 name=f"h1ps{h}") for h in range(2)]
                XMv = XM[:].rearrange("p n (m j i) -> p m i j n",
                                      m=5, j=2, i=16)
                for t in range(NPAIR):
                    if t < 80:
                        lhsT = XMv[:, t // 16, t % 16, :, :]
                    else:
                        lhsT = X_cp[:, :, :]
                    for h in range(2):
                        nc.tensor.matmul(h1ps[h][:], lhsT,
                                         w1v[:, t, :, 324 * h:324 * h + 324],
                                         start=(t == 0), stop=False,
                                         perf_mode=DR)
                for h in range(2):
                    nc.tensor.matmul(h1ps[h][:], onesrow[:],
                                     b1row[:, 324 * h:324 * h + 324],
                                     start=False, stop=True,
                                     skip_group_check=True)
                for h in range(2):
                    nc.scalar.activation(h1sb[:, 324 * h:324 * h + 324],
                                         h1ps[h][:], AF.Relu,
                                         scale=1.0 / 512.0)

            if dbg:
                nc.sync.dma_start(dh1_d[:, :], h1sb[:])
            # transpose h1 -> fp8 pairs [128, 6, 64] (649..768 zero)
            h1T = persist.tile([128, 6, N], dt.float8e4)
            nc.vector.memset(h1T[:], 0.0)
            with tc.tile_pool(name="ptp", bufs=2, space="PSUM") as ptp:
                for u in range(6):
                    w = 128 if u < 5 else COLS1 - 5 * 128  # 8
                    tp = ptp.tile([128, 64], dt.bfloat16, tag="tp")
                    nc.tensor.transpose(tp[0:w, :], h1sb[:, 128 * u:128 * u + w],
                                        ident[0:64, 0:64])
                    nc.vector.tensor_copy(h1T[0:w, u, :], tp[0:w, :])

            # ---------------- L2 partial (DoubleRow) + AllToAll ------------
            w2v = w2sb[:].rearrange("p (t c) -> p t c", t=6)
            with tc.tile_pool(name="l2", bufs=1) as l2p, \
                 tc.tile_pool(name="ph2", bufs=1, space="PSUM") as ph2:
                h2ps = [ph2.tile([64, 432], dt.float32, tag=f"h2ps{h}",
                                 name=f"h2ps{h}") for h in range(3)]
                for u in range(3):
                    for h in range(3):
                        nc.tensor.matmul(
                            h2ps[h][:], h1T[:, 2 * u:2 * u + 2, :],
                            w2v[:, 2 * u:2 * u + 2, 432 * h:432 * h + 432],
                            start=(u == 0), stop=False, perf_mode=DR)
                for h in range(3):
                    nc.tensor.matmul(h2ps[h][:], onesrow[:],
                                     b2row[:, 432 * h:432 * h + 432],
                                     start=False, stop=True,
                                     skip_group_check=True)
                h2bf = l2p.tile([64, D2], dt.bfloat16, tag="h2bf")
                for h in range(3):
                    nc.scalar.activation(h2bf[:, 432 * h:432 * h + 432],
                                         h2ps[h][:], AF.Copy,
                                         scale=1.0 / 262144.0)
                if dbg:
                    nc.sync.dma_start(dh2_d[:, :], h2bf[:])
                nc.sync.dma_start(a2a_in[:, :], h2bf[:])
                nc.gpsimd.collective_compute(
                    "AllToAll", mybir.AluOpType.bypass, replica_groups=rg,
                    ins=[a2a_in[:]], outs=[a2a_out[:]])

            # ---------------- L3 on this core's 8 batches ----------------
            h2T = persist.tile([128, 11, NB], dt.bfloat16)
            nc.vector.memset(h2T[:], 0.0)
            h3sb = persist.tile([NB, D3], dt.bfloat16)
            with tc.tile_pool(name="l3", bufs=1) as l3p, \
                 tc.tile_pool(name="ptp2", bufs=2, space="PSUM") as ptp2, \
                 tc.tile_pool(name="ph3s", bufs=1, space="PSUM") as ph3s, \
                 tc.tile_pool(name="ph3", bufs=1, space="PSUM") as ph3:
                a2sb = l3p.tile([N, D2], dt.bfloat16, tag="a2sb")
                nc.sync.dma_start(a2sb[:], a2a_out[:, :])
                h2r = l3p.tile([NB, D2], dt.bfloat16, tag="h2r")
                for h in range(3):
                    hp = ph3s.tile([NB, 432], dt.float32, tag=f"h2s{h}",
                                   name=f"h2s{h}")
                    nc.tensor.matmul(hp[:], selT[:],
                                     a2sb[:, 432 * h:432 * h + 432],
                                     start=True, stop=True)
                    nc.scalar.activation(h2r[:, 432 * h:432 * h + 432],
                                         hp[:], AF.Relu)
                for t in range(11):
                    w = 128 if t < 10 else D2 - 10 * 128  # 16
                    tp = ptp2.tile([128, NB], dt.bfloat16, tag="tp2")
                    nc.tensor.transpose(tp[0:w, :], h2r[:, 128 * t:128 * t + w],
                                        ident[0:NB, 0:NB])
                    nc.vector.tensor_copy(h2T[0:w, t, :], tp[0:w, :])
                h3ps = ph3.tile([NB, D3], dt.float32, tag="h3ps")
                for t in range(11):
                    nc.tensor.matmul(h3ps[:], h2T[:, t, :], w3sb[:, t, :],
                                     start=(t == 0), stop=False)
                nc.tensor.matmul(h3ps[:], onesrow[:, 0:NB], b3row[:],
                                 start=False, stop=True)
                nc.scalar.activation(h3sb[:], h3ps[:], AF.Tanh)

            # ---------------- L4 ----------------
            h3T = persist.tile([128, 3, NB], dt.bfloat16)
            nc.vector.memset(h3T[:], 0.0)
            with tc.tile_pool(name="ptp3", bufs=2, space="PSUM") as ptp3, \
                 tc.tile_pool(name="l4", bufs=1) as l4p, \
                 tc.tile_pool(name="ph4", bufs=1, space="PSUM") as ph4:
                for t in range(3):
                    w = 128 if t < 2 else D3 - 256  # 68
                    tp = ptp3.tile([128, NB], dt.bfloat16, tag="tp3")
                    nc.tensor.transpose(tp[0:w, :], h3sb[:, 128 * t:128 * t + w],
                                        ident[0:NB, 0:NB])
                    nc.vector.tensor_copy(h3T[0:w, t, :], tp[0:w, :])
                outps = ph4.tile([NB, 8], dt.float32, tag="outps")
                for t in range(3):
                    nc.tensor.matmul(outps[:], h3T[:, t, :], w4sb[:, t, :],
                                     start=(t == 0), stop=False)
                nc.tensor.matmul(outps[:], onesrow[:, 0:NB], b4row[:],
                                 start=False, stop=True)
                outsb = l4p.tile([NB, 8], dt.float32, tag="outsb")
                nc.vector.tensor_copy(outsb[:], outps[:])
                nc.sync.dma_start(out_d[:, :], outsb[:])

    nc.compile()
    return nc


def _build_w1m(w1core):
    """w1core: [20736, 648] fp32 (already * 1024).
    Returns [128, NPAIR*2*PADC] fp8 in the pair-tile layout."""
    out = np.zeros((128, NPAIR * 2 * PADC), dtype=ml_dtypes.float8_e4m3)
    q = _fp8(w1core)
    kt = q.reshape(HW, HW, COLS1)        # [k, ij, col]
    # pairs t<80: kappa = 32*(t//16) + 16*j + t%16
    #   kappa < 144: main tile  kk = kappa*144 + p      (p = ij 0:128)
    #   kappa >=144: swres tile kk = p*144 + 128 + (kappa-144)  (p = k)
    for t in range(80):
        m, i = t // 16, t % 16
        for j in range(2):
            kappa = 32 * m + 16 * j + i
            base = t * 2 * PADC + j * PADC
            if kappa < HW:
                out[:, base:base + COLS1] = kt[kappa, 0:128, :]
            else:
                out[:, base:base + COLS1] = kt[0:128, 128 + (kappa - HW), :]
    # corner t=80: p = 16*il + kr ; kk = (128+kr)*144 + 128 + (8j+il)
    t = 80
    for j in range(2):
        base = t * 2 * PADC + j * PADC
        for il in range(8):
            for kr in range(16):
                out[16 * il + kr, base:base + COLS1] = \
                    kt[128 + kr, 128 + 8 * j + il, :]
    return out


def _prep_inputs(x1, x2, w1, b1, w2, b2, w3, b3, w4, b4):
    x1f = np.asarray(x1, np.float32).reshape(N, C, HW)
    x2f = np.asarray(x2, np.float32).reshape(N, C, HW)
    x1q = _bf16(np.ascontiguousarray(x1f.transpose(1, 0, 2)))
    x2q = _bf16(np.ascontiguousarray(x2f.transpose(1, 0, 2)))
    w1 = np.asarray(w1, np.float32)
    w2 = np.asarray(w2, np.float32)
    w3 = np.asarray(w3, np.float32)
    w4 = np.asarray(w4, np.float32)
    b1 = np.asarray(b1, np.float32)
    b2 = np.asarray(b2, np.float32)

    w3pad = np.zeros((PADK3, D3), np.float32)
    w3pad[:D2] = w3
    w4pad = np.zeros((PADK4, 8), np.float32)
    w4pad[:D3] = w4
    w3b = _bf16(w3pad)
    w4b = _bf16(w4pad)
    b2f = _bf16(b2 / NCORES * 262144.0).reshape(1, D2)
    b3r = _bf16(b3).reshape(1, D3)
    b4r = _bf16(b4).reshape(1, 8)

    in_maps = []
    for core in range(NCORES):
        w1c = w1[:, COLS1 * core:COLS1 * (core + 1)] * 1024.0
        w1m = _build_w1m(w1c)
        # w2 rows for this core's h1 slice, *1024, padded to 768, [p, t*c]
        w2c = np.zeros((768, D2), np.float32)
        w2c[:COLS1] = w2[COLS1 * core:COLS1 * (core + 1)] * 1024.0
        w2m = _fp8(np.ascontiguousarray(
            w2c.reshape(6, 128, D2).transpose(1, 0, 2).reshape(128, 6 * D2)))
        in_maps.append({
            "x1q": x1q, "x2q": x2q,
            "w1m": w1m,
            "b1s": _bf16(b1[COLS1 * core:COLS1 * (core + 1)]
                         * 131072.0).reshape(1, COLS1),
            "w2m": w2m,
            "b2f": b2f,
            "w3f": w3b,
            "b3r": b3r,
            "w4p": w4b,
            "b4r": b4r,
        })
    return in_maps


def kernel(x1, x2, w1, b1, w2, b2, w3, b3, w4, b4):
    global LAST_RESULT
    from concourse.bass_utils import run_bass_kernel_spmd

    if "nc" not in _CACHE:
        _CACHE["nc"] = _build_nc()
    nc = _CACHE["nc"]

    in_maps = _prep_inputs(x1, x2, w1, b1, w2, b2, w3, b3, w4, b4)
    trace = bool(int(os.environ.get("HNET_TRACE", "0")))
    res = run_bass_kernel_spmd(nc, in_maps, core_ids=list(range(NCORES)),
                               trace=trace)
    LAST_RESULT = res
    H = np.concatenate(
        [np.asarray(res.results[c]["out"], np.float32) for c in range(NCORES)],
        axis=0)
    ones = np.ones((N, 1), np.float32)
    return np.concatenate([H, ones], axis=1).reshape(N, 3, 3)
